# revision 26
# baseline (speedup 1.0000x reference)
"""Multi-head attention (nn_MultiHeadAttention) on 8 Trainium2 NeuronCores.

Hybrid batch x head sharding: core c owns batch c//4 and heads
4*(c%4)..4*(c%4)+3 (two head-PAIRS). Each core computes its 4 heads' full
attention plus the partial output projection for its batch; the host sums
the 4 partials per batch and adds bo.

Schedule: the Scalar engine's exp stream (16.8M elems, ~135us at
0.83ns/col) is the co-critical resource with the PE (~150us of matmul
rows). The kernel starts the exp stream as early as possible (khT pair-0
projection first, then qhT half-0, first scores at ~13us) and
redistributes every other PE obligation (remaining q/k projections,
pair-1 attention prep, v-projection, output projection) as slack work
interleaved INSIDE the exp-bound supersteps, so both engines run
back-to-back to the end.

Per-core kernel phases:
  L   DMA: wk/wq first; kT split across the Sync+Act DGE queues (lands
      ~11us); qT behind kT on Act; vT on the Vector DGE queue from t=0;
      everything else (wo/bv/consts) on the GpSimd queue.
  P1  khT pair-0 (all 4 halves) then qhT pair-0 half-0 only -> first
      scores/exp tile starts while the other 11 projection halves ride
      the superstep slack.
  A   software-pipelined attention: scores+exp for tile (p, sq+1)
      interleave with attn@V of tile (p, sq). scoresT = khT.T@qhT packs
      both heads of a pair via tile_position; attn@V carries a 65th
      denominator row (constant 128.0 column in vh implements the
      softmax/(2*dk) scaling exactly; 2*DK == 128). Deferred
      normalization: reciprocal_approx_fast (after a partition-64->0 DMA
      hop; the custom DVE op mis-executes on partition bases >= 64) + PE
      broadcast + DVE mul, deferred one superstep so the latency hides.
      vproj pair-0 fills the prologue; vproj pair-1 and the projection
      halves ride supersteps 1-4; outproj tiles ride supersteps 6-8 as
      their cat columns + reciprocals become final.
  O   joint output projection: po = cat0.T@wo0 + cat1.T@wo1 accumulated
      in PSUM (K=128 per pair), PSUM->SBUF copies on DVE, bf16 partials
      summed on host.
"""

from contextlib import ExitStack

import numpy as np
import ml_dtypes

import concourse.bass as bass
import concourse.tile as tile
from concourse import bacc
from concourse import mybir

F32 = mybir.dt.float32
F32R = mybir.dt.float32r
BF16 = mybir.dt.bfloat16
EXP = mybir.ActivationFunctionType.Exp

B, S, D, NH, DK, DV = 2, 2048, 1024, 16, 64, 64
NCORES = 8
HPC = 4          # heads per core
NPAIR = 2        # head pairs per core


def build_nc(s=S, d=D):
    """Build the per-core Bass program (identical on all 8 cores)."""
    nc = bacc.Bacc("TRN2", target_bir_lowering=False, debug=False)

    sq_t = 512                  # sq tile (matmul free dim)
    n_sq = s // sq_t
    n_sk = s // 128             # sk tiles of 128
    n_ch = d // 128             # contraction chunks of 128

    qT_d = nc.dram_tensor("qT", [d, s], BF16, kind="ExternalInput").ap()
    kT_d = nc.dram_tensor("kT", [d, s], BF16, kind="ExternalInput").ap()
    vT_d = nc.dram_tensor("vT", [d, s], BF16, kind="ExternalInput").ap()
    wq_d = nc.dram_tensor("wq", [NPAIR, d, 128], BF16, kind="ExternalInput").ap()
    wk_d = nc.dram_tensor("wk", [NPAIR, d, 128], BF16, kind="ExternalInput").ap()
    wv_d = nc.dram_tensor("wv", [NPAIR, d, 128], BF16, kind="ExternalInput").ap()
    bqk_d = nc.dram_tensor("bqk", [128, 2 * NPAIR], F32, kind="ExternalInput").ap()
    bv_d = nc.dram_tensor("bv_col", [128, NPAIR], F32, kind="ExternalInput").ap()
    wo_d = nc.dram_tensor("wo", [NPAIR, 128, d], BF16, kind="ExternalInput").ap()
    onesr_d = nc.dram_tensor("onesr", [128, 64], F32R, kind="ExternalInput").ap()
    ident_d = nc.dram_tensor("ident", [128, 128], BF16, kind="ExternalInput").ap()
    out_d = nc.dram_tensor("out", [s, d], BF16, kind="ExternalOutput").ap()

    with tile.TileContext(nc) as tc, ExitStack() as ctx:
        consts = ctx.enter_context(tc.tile_pool(name="consts", bufs=1))
        qk_sb = ctx.enter_context(tc.tile_pool(name="qk_sb", bufs=1))
        vt_stream = ctx.enter_context(tc.tile_pool(name="vt_stream", bufs=12))
        vhT_pool = ctx.enter_context(tc.tile_pool(name="vhT", bufs=3))
        qkt_pool = ctx.enter_context(tc.tile_pool(name="qkt", bufs=1))
        vh_pool = ctx.enter_context(tc.tile_pool(name="vh", bufs=1))
        exp_pool = ctx.enter_context(tc.tile_pool(name="expp", bufs=16))
        cat_pool = ctx.enter_context(tc.tile_pool(name="cat", bufs=1))
        recip_pool = ctx.enter_context(tc.tile_pool(name="recip", bufs=2))
        out_pool = ctx.enter_context(tc.tile_pool(name="outp", bufs=2))
        # PSUM (8 banks): sAB double-buffer 4 + nA/nB accumulators 2 +
        # everything else (proj/vproj/outproj/bcast tiles) rotating in 2.
        # nA/nB live across a whole k-loop, so they get a dedicated tag --
        # any other tile reusing their banks mid-k-loop would wait on a
        # drain that sits BEHIND it in the in-order PE queue (deadlock).
        ps = ctx.enter_context(tc.tile_pool(name="ps", bufs=2, space="PSUM"))

        # --- weights that gate the k projection go first, split across the
        # two queues that also carry kT so either queue unblocks it ---
        wq_sb = consts.tile([128, NPAIR, n_ch, 128], BF16, tag="wq")
        wk_sb = consts.tile([128, NPAIR, n_ch, 128], BF16, tag="wk")
        wv_sb = consts.tile([128, NPAIR, n_ch, 128], BF16, tag="wv")
        nc.sync.dma_start(
            wk_sb[:, 0], wk_d[0].rearrange("(c p) m -> p c m", p=128))
        nc.scalar.dma_start(
            wq_sb[:, 0], wq_d[0].rearrange("(c p) m -> p c m", p=128))
        bqk_sb = consts.tile([128, 2 * NPAIR], F32, tag="bqk")
        nc.sync.dma_start(bqk_sb[:], bqk_d[:])

        # ---- Phase L. DMA triggers execute IN-ORDER on their dispatching
        # engine, and the Act engine also runs the exp stream -- so the
        # scalar ring gets only the handful of transfers that gate the
        # first scores (kt-odd chunks, qt h0/h1 even chunks) and nothing
        # after. Everything else: sync ring (kt-even, qt rest, den hops,
        # output tiles) and the gpsimd software DGE (consts, vT blocks).
        # Transfers are batched (one trigger per half/parity) to keep
        # trigger counts tiny.
        qt_sb = qk_sb.tile([128, n_ch, s], BF16, tag="qt")
        kt_sb = qk_sb.tile([128, n_ch, s], BF16, tag="kt")

        # One dma_start executes on ONE DMA engine (~22.5 GB/s), so each
        # half stays split into per-chunk transfers for engine-level
        # parallelism. The scalar (Act) ring gets only 4 of them -- its
        # trigger queue must drain before the exp stream starts.
        def emit_half(sbuf, dram, hs, chunks, eng):
            ssl = bass.ts(hs, sq_t)
            for c in chunks:
                csl = slice(c * 128, (c + 1) * 128)
                eng.dma_start(sbuf[:, c, ssl], dram[csl, ssl])

        emit_half(kt_sb, kT_d, 0, range(6), nc.sync)
        emit_half(kt_sb, kT_d, 0, (6, 7), nc.scalar)
        nc.sync.dma_start(
            wk_sb[:, 1], wk_d[1].rearrange("(c p) m -> p c m", p=128))
        nc.scalar.dma_start(
            wq_sb[:, 1], wq_d[1].rearrange("(c p) m -> p c m", p=128))
        for hs in range(1, n_sq):
            emit_half(kt_sb, kT_d, hs, range(n_ch), nc.sync)
        emit_half(qt_sb, qT_d, 0, range(6), nc.sync)
        emit_half(qt_sb, qT_d, 0, (6, 7), nc.scalar)
        # the rest of the sync stream is emitted below in strict
        # consumption-deadline order, interleaved with the prologue vT
        # t-blocks (the early phase is DMA-bandwidth-bound)

        # v-side constants + vT blocks ride the otherwise-idle GpSimd
        # software-DGE queue (high per-transfer overhead, zero contention
        # with the two hardware rings). Small consts first (they gate the
        # prologue vproj), then the four vT t-blocks, then wo.
        bv_sb = consts.tile([128, NPAIR], F32, tag="bv")
        nc.gpsimd.dma_start(bv_sb[:], bv_d[:])
        ident = consts.tile([128, 128], BF16, tag="ident")
        nc.gpsimd.dma_start(ident[:], ident_d[:])
        for p in range(NPAIR):
            nc.gpsimd.dma_start(
                wv_sb[:, p], wv_d[p].rearrange("(c p) m -> p c m", p=128))
        ones_fr = consts.tile([128, 64], F32R, tag="ones_fr")
        nc.gpsimd.dma_start(ones_fr[:], onesr_d[:])

        vt2 = {}

        def emit_vt_dmas(tt, eng):
            tsl = bass.ts(tt, 512)
            vt_c = vt_stream.tile([128, n_ch, 512], BF16, tag="vt", bufs=2)
            src = vT_d[:, tsl].rearrange("(c p) s -> p c s", p=128)
            for c in range(n_ch):
                eng.dma_start(vt_c[:, c, :], src[:, c, :])
            return vt_c

        # deadline order on sync: vt-tt0/tt1 (prologue vproj) before
        # qt-h1 (prologue-end projection), then tt2/tt3, then qt h2/h3
        vt2[0] = emit_vt_dmas(0, nc.sync)
        vt2[1] = emit_vt_dmas(1, nc.sync)
        emit_half(qt_sb, qT_d, 1, range(n_ch), nc.sync)
        vt2[2] = emit_vt_dmas(2, nc.sync)
        vt2[3] = emit_vt_dmas(3, nc.sync)
        emit_half(qt_sb, qT_d, 2, range(n_ch), nc.sync)
        emit_half(qt_sb, qT_d, 3, range(n_ch), nc.sync)

        wo_sb = consts.tile([128, NPAIR, d], BF16, tag="wo")
        for p in range(NPAIR):
            nc.gpsimd.dma_start(wo_sb[:, p, :], wo_d[p])

        # ---- qhT/khT head-projection targets
        qhTs, khTs = [], []
        for p in range(NPAIR):
            qhT = qkt_pool.tile([128, s], BF16, tag=f"qhT{p}")
            khT = qkt_pool.tile([128, s], BF16, tag=f"khT{p}")
            qhTs.append(qhT)
            khTs.append(khT)

        def emit_scores_exp(qhT, khT, sq, k):
            ssl = bass.ts(sq, sq_t)
            ksl = bass.ts(k, 128)
            sAB = ps.tile([128, 2 * sq_t], F32, tag="ps2", bufs=2)
            nc.tensor.matmul(sAB[:, 0:sq_t], khT[0:64, ksl], qhT[0:64, ssl],
                             start=True, stop=True, tile_position=(0, 0))
            nc.tensor.matmul(sAB[:, sq_t:2 * sq_t], khT[64:128, ksl],
                             qhT[64:128, ssl],
                             start=True, stop=True, tile_position=(64, 0))
            eAB = exp_pool.tile([128, 2 * sq_t], BF16, tag="eAB")
            nc.scalar.activation(eAB[:], sAB[:], EXP)
            return eAB

        # vh layout per k-tile: [vhA+bvA (64) | 128.0 | pad | vhB+bvB | 128.0 | pad]
        vhs = []
        for p in range(NPAIR):
            vh = vh_pool.tile([128, n_sk, 132], BF16, tag=f"vh{p}", name=f"vh{p}")
            # constant softmax-denominator columns (128 == 2*DK scaling)
            nc.gpsimd.memset(vh[:, :, 64:65], 128.0)
            nc.gpsimd.memset(vh[:, :, 130:131], 128.0)
            vhs.append(vh)

        def emit_vproj(p, tt, vt_c):
            psv = ps.tile([128, 512], F32, tag="ps", name=f"psv{p}")
            for c in range(n_ch):
                nc.tensor.matmul(psv[:], wv_sb[:, p, c, :], vt_c[:, c, :],
                                 start=(c == 0), stop=(c == n_ch - 1))
            vsb = vhT_pool.tile([128, 512], BF16, tag="vsb")
            with nc.allow_low_precision(reason="bf16 rounding as baseline"):
                nc.vector.tensor_scalar_add(vsb[:], psv[:], bv_sb[:, p:p + 1])
            for j in range(4):
                k = tt * 4 + j
                # PE transpose [dv2, t128] -> [t128, dv2], writing the
                # gap layout (cols 0:64 head A, 65:129 head B); the
                # constant denominator columns are memset once above.
                vtr = ps.tile([128, 132], BF16, tag="ps", name="vtr")
                tr_dst = vtr.rearrange("p (b c) -> p b c", b=2)[:, :, 0:64]
                nc.tensor.transpose(tr_dst, vsb[:, j * 128:(j + 1) * 128],
                                    ident[:])
                dst = vhs[p][:, k, :].rearrange(
                    "p (b c) -> p b c", b=2)[:, :, 0:64]
                src = vtr.rearrange("p (b c) -> p b c", b=2)[:, :, 0:64]
                nc.vector.tensor_copy(dst, src)

        def emit_proj_q(p, half):
            ssl = bass.ts(half, sq_t)
            pq = ps.tile([128, sq_t], F32, tag="ps", name=f"psq{p}_{half}")
            for c in range(n_ch):
                nc.tensor.matmul(pq[:], wq_sb[:, p, c, :], qt_sb[:, c, ssl],
                                 start=(c == 0), stop=(c == n_ch - 1))
            with nc.allow_low_precision(reason="bf16 rounding as baseline"):
                nc.vector.tensor_scalar_add(qhTs[p][:, ssl], pq[:],
                                            bqk_sb[:, 2 * p:2 * p + 1])

        def emit_proj_k(p, half):
            ssl = bass.ts(half, sq_t)
            pk = ps.tile([128, sq_t], F32, tag="ps", name=f"psk{p}_{half}")
            for c in range(n_ch):
                nc.tensor.matmul(pk[:], wk_sb[:, p, c, :], kt_sb[:, c, ssl],
                                 start=(c == 0), stop=(c == n_ch - 1))
            with nc.allow_low_precision(reason="bf16 rounding as baseline"):
                nc.vector.tensor_scalar_add(khTs[p][:, ssl], pk[:],
                                            bqk_sb[:, 2 * p + 1:2 * p + 2])

        # ---- Phase P1: only what gates the first exp tile: khT pair-0
        # (all halves) then qhT pair-0 half-0.
        for half in range(n_sq):
            emit_proj_k(0, half)
        emit_proj_q(0, 0)

        # Slack-work queue: the projection halves the PE still owes,
        # consumed up to 4 units per superstep. The PE queue is in-order,
        # so a half must be EMITTED a full superstep before the scores
        # that read it (superstep i emits scores for tile i+1): qhT p0
        # half h by superstep h-1, khT p1 + qhT p1 h0 by superstep 3,
        # qhT p1 half h by superstep 4+h-1. (qhT p0 h1 is emitted in the
        # prologue for the same reason.)
        slack = [
            lambda: emit_proj_q(0, 2),
            lambda: emit_proj_k(1, 0),
            lambda: emit_proj_k(1, 1),
            lambda: emit_proj_k(1, 2),
            lambda: emit_proj_q(0, 3),
            lambda: emit_proj_k(1, 3),
            lambda: emit_proj_q(1, 0),
            lambda: emit_proj_q(1, 1),
            lambda: emit_proj_q(1, 2),
            lambda: emit_proj_q(1, 3),
        ]

        cats = []
        for p in range(NPAIR):
            cats.append(cat_pool.tile([128, s], BF16, tag=f"cat{p}", name=f"cat{p}"))

        def emit_outproj(ot):
            osl = bass.ts(ot, 128)
            o_sb = out_pool.tile([128, d], BF16, tag="o", name=f"o{ot}")
            mo = None
            for dh in range(2):
                dsl = bass.ts(dh, 512)
                po = ps.tile([128, 512], F32, tag="ps", name=f"po{ot}_{dh}")
                nc.tensor.matmul(po[:], cats[0][:, osl], wo_sb[:, 0, dsl],
                                 start=True, stop=False)
                mo = nc.tensor.matmul(po[:], cats[1][:, osl], wo_sb[:, 1, dsl],
                                      start=False, stop=True)
                with nc.allow_low_precision(reason="bf16 partials, host sum"):
                    nc.vector.tensor_copy(o_sb[:, dsl], po[:])
            nc.sync.dma_start(out_d[ot * 128:(ot + 1) * 128, :], o_sb[:])
            return mo

        # ---- Phase A: prologue = scores+exp (p0, sq0) interleaved with
        # the full pair-0 v-projection (vT arrives tt-progressively on
        # the DGE rings behind qt-h0), then qhT p0 h1 (whose scores are
        # emitted from superstep 1's k=0).
        vt2b = {}
        eABs = []
        for k in range(n_sk):
            eABs.append(emit_scores_exp(qhTs[0], khTs[0], 0, k))
            if k % 4 == 3:
                emit_vproj(0, k // 4, vt2[k // 4])
        emit_proj_q(0, 1)

        tiles = [(p, sq) for p in range(NPAIR) for sq in range(n_sq)]
        pending_norm = None
        # outproj group g (= sq) becomes legal once norm(p1, sq) has been
        # EMITTED; norms are deferred one superstep, so norm(p1, s0) is
        # emitted after superstep 6's k-loop -> group 0 rides superstep 7.
        outproj_ready = {7: [0, 1, 2, 3], 8: [4, 5, 6, 7]}
        outproj_q = []

        for ti, (p, sq) in enumerate(tiles):
            step = ti + 1
            qhT, khT, vh, cat = qhTs[p], khTs[p], vhs[p], cats[p]
            nxt = tiles[ti + 1] if ti + 1 < len(tiles) else None
            ssl = bass.ts(sq, sq_t)
            if p == 0:
                vt2b[sq] = emit_vt_dmas(sq, nc.gpsimd)
            outproj_q.extend(outproj_ready.get(step, []))
            nA = ps.tile([128, sq_t], F32, tag="psn", bufs=2)
            nB = ps.tile([128, sq_t], F32, tag="psn", bufs=2)
            anchor = None
            next_eABs = []
            for k in range(n_sk):
                if nxt is not None:
                    next_eABs.append(
                        emit_scores_exp(qhTs[nxt[0]], khTs[nxt[0]],
                                        nxt[1], k))
                eAB = eABs[k]
                nc.tensor.matmul(nA[0:65, :], vh[:, k, 0:65], eAB[:, 0:sq_t],
                                 start=(k == 0), stop=(k == n_sk - 1))
                mm_b = nc.tensor.matmul(nB[0:65, :], vh[:, k, 66:131],
                                        eAB[:, sq_t:2 * sq_t],
                                        start=(k == 0),
                                        stop=(k == n_sk - 1))
                if k == min(8, n_sk - 1):
                    anchor = mm_b
                # slack work: a few units per superstep keep the PE dense
                # while the projection/vproj/outproj backlog drains, paced
                # so the scores->exp stream stays ahead of the Scalar
                # engine.
                if p == 0 and k == 7:
                    emit_vproj(1, sq, vt2b[sq])
                elif k in (1, 5, 11) and slack:
                    slack.pop(0)()
                elif k % 4 == 3 and outproj_q:
                    emit_outproj(outproj_q.pop(0))
            eABs = next_eABs
            if pending_norm is not None:
                pending_norm(anchor)
                pending_norm = None
            # free nA/nB quickly: copy numerators + denominators out of
            # PSUM before the reciprocal runs.
            numAB = recip_pool.tile([64, 2 * sq_t], F32, tag="numAB")
            nc.vector.tensor_copy(numAB[:, 0:sq_t], nA[0:64, :])
            nc.vector.tensor_copy(numAB[:, sq_t:2 * sq_t], nB[0:64, :])
            den64 = recip_pool.tile([65, 2 * sq_t], F32, tag="den64")
            nc.vector.tensor_copy(den64[64:65, 0:sq_t], nA[64:65, :])
            nc.vector.tensor_copy(den64[64:65, sq_t:2 * sq_t], nB[64:65, :])
            rec = recip_pool.tile([1, 4 * sq_t], F32, tag="rec")
            # SBUF->SBUF partition move 64 -> 0: reciprocal_approx_fast
            # mis-executes on partition bases >= 64
            nc.sync.dma_start(rec[0:1, 0:2 * sq_t], den64[64:65, :])
            nc.vector.reciprocal_approx_fast(
                rec[0:1, 2 * sq_t:4 * sq_t], rec[0:1, 0:2 * sq_t])
            recr = recip_pool.tile([1, 2 * sq_t], F32R, tag="recr")
            with nc.allow_low_precision(reason="f32r == f32 bits"):
                nc.vector.tensor_copy(recr[0:1, :],
                                      rec[0:1, 2 * sq_t:4 * sq_t])

            def _normalize(anc, ssl=ssl, recr=recr, numAB=numAB, cat=cat):
                # deferred one sq-tile so the reciprocal latency hides
                # under the next k-loop instead of stalling the PE queue
                bcA = ps.tile([128, sq_t], F32, tag="ps", name="bcA")
                bcB = ps.tile([128, sq_t], F32, tag="ps", name="bcB")
                mA = nc.tensor.matmul(
                    bcA[0:64, :], ones_fr[0:1, :],
                    recr[0:1, 0:sq_t],
                    start=True, stop=True)
                if anc is not None:
                    tile.add_dep_helper(mA.ins, anc.ins, sync=False,
                                        reason="defer bcast past k-loop")
                nc.tensor.matmul(bcB[0:64, :], ones_fr[0:1, :],
                                 recr[0:1, sq_t:2 * sq_t],
                                 start=True, stop=True)
                nc.vector.tensor_mul(cat[0:64, ssl], bcA[0:64, :],
                                     numAB[:, 0:sq_t])
                nc.vector.tensor_mul(cat[64:128, ssl], bcB[0:64, :],
                                     numAB[:, sq_t:2 * sq_t])
            pending_norm = _normalize

        # ---- Phase O: whatever output projection hasn't ridden the
        # superstep slack; only ot 12..15 depend on the last normalization
        o_anchor = None
        n_ot = s // 128
        outproj_q.extend(range(8, n_ot))
        n_early = sum(1 for ot in outproj_q if ot < 12)
        for i, ot in enumerate(outproj_q):
            if pending_norm is not None and ot == 12:
                pending_norm(o_anchor)
                pending_norm = None
            mo = emit_outproj(ot)
            if i == max(0, n_early - 2):
                o_anchor = mo
        if pending_norm is not None:
            pending_norm(None)
            pending_norm = None

    nc.compile()
    return nc


def make_core_inputs(Q, K, V, Wq, bq, Wk, bk, Wv, bv, Wo):
    """Host-side prep: transposes, casts, per-core weight packing."""
    bf = ml_dtypes.bfloat16
    QT = np.ascontiguousarray(
        np.transpose(np.asarray(Q, np.float32), (0, 2, 1))).astype(bf)
    KT = np.ascontiguousarray(
        np.transpose(np.asarray(K, np.float32), (0, 2, 1))).astype(bf)
    VT = np.ascontiguousarray(
        np.transpose(np.asarray(V, np.float32), (0, 2, 1))).astype(bf)

    in_maps = []
    for c in range(NCORES):
        bi = c // 4
        h0 = HPC * (c % 4)
        wq = np.stack([np.concatenate([Wq[h0 + 2 * p], Wq[h0 + 2 * p + 1]], 1)
                       for p in range(NPAIR)]).astype(np.float32).astype(bf)
        wk = np.stack([np.concatenate([Wk[h0 + 2 * p], Wk[h0 + 2 * p + 1]], 1)
                       for p in range(NPAIR)]).astype(np.float32).astype(bf)
        wv = np.stack([np.concatenate([Wv[h0 + 2 * p], Wv[h0 + 2 * p + 1]], 1)
                       for p in range(NPAIR)]).astype(np.float32).astype(bf)
        bqk = np.stack(
            [np.concatenate([bq[h0 + 2 * p], bq[h0 + 2 * p + 1]])
             if col == 0 else
             np.concatenate([bk[h0 + 2 * p], bk[h0 + 2 * p + 1]])
             for p in range(NPAIR) for col in range(2)],
            axis=1).astype(np.float32)
        bvc = np.stack(
            [np.concatenate([bv[h0 + 2 * p], bv[h0 + 2 * p + 1]])
             for p in range(NPAIR)], axis=1).astype(np.float32)
        wo = np.stack(
            [np.concatenate([Wo[64 * (h0 + 2 * p):64 * (h0 + 2 * p) + 64],
                             Wo[64 * (h0 + 2 * p + 1):64 * (h0 + 2 * p + 1) + 64]],
                            0)
             for p in range(NPAIR)]).astype(np.float32).astype(bf)
        in_maps.append({
            "qT": QT[bi], "kT": KT[bi], "vT": VT[bi],
            "wq": wq, "wk": wk, "wv": wv,
            "bqk": bqk, "bv_col": bvc, "wo": wo,
            "onesr": np.ones((128, 64), np.float32),
            "ident": np.eye(128, dtype=np.float32).astype(bf),
        })
    return in_maps


_NC_CACHE = {}


def _get_nc():
    if "nc" not in _NC_CACHE:
        _NC_CACHE["nc"] = build_nc()
    return _NC_CACHE["nc"]


def _install_ntff_hook_shim():
    """The agent image's antenv lacks axon_hooks; recreate the tiny
    get/set registry and register the ctypes NTFF profiler so trace=True
    can report HW exec time."""
    import sys
    import types
    if "antenv.axon_hooks" in sys.modules:
        return
    hook = None
    try:
        from trn_agent_boot.trn_boot import _ntff_profile_via_ctypes
        hook = _ntff_profile_via_ctypes("/opt/axon/libaxon_pjrt.so")
    except Exception:
        hook = None
    mod = types.ModuleType("antenv.axon_hooks")
    mod._hook = hook
    mod.get_axon_ntff_profile_hook = lambda: mod._hook
    mod.set_axon_ntff_profile_hook = lambda h: setattr(mod, "_hook", h)
    sys.modules["antenv.axon_hooks"] = mod


def kernel(Q, K, V, Wq, bq, Wk, bk, Wv, bv, Wo, bo, _trace=False):
    from concourse.bass_utils import run_bass_kernel_spmd

    if _trace:
        _install_ntff_hook_shim()

    nc = _get_nc()
    in_maps = make_core_inputs(Q, K, V, Wq, bq, Wk, bk, Wv, bv, Wo)
    res = None
    for attempt in range(3):
        try:
            res = run_bass_kernel_spmd(nc, in_maps, list(range(NCORES)),
                                       trace=_trace)
            break
        except Exception:
            # transient NRT_EXEC_UNIT_UNRECOVERABLE wedges recover on retry
            if attempt == 2:
                raise
    out = np.zeros((B, S, D), np.float32)
    for c, r in enumerate(res.results):
        out[c // 4] += np.asarray(r["out"]).astype(np.float32)
    out += np.asarray(bo, np.float32)[None, None, :]
    if _trace:
        return out, res
    return out
